# revision 32
# baseline (speedup 1.0000x reference)
"""Trainium2 Bass kernel for Mixtral-style GQA attention.

Full module: y = Attn(RoPE(hs@Wq), RoPE(hs@Wk), hs@Wv) @ Wo
  T=2048, HIDDEN=4096, 32 Q heads / 8 KV heads, head_dim=128, causal,
  neox rotate-half RoPE (base 1e6), fp32 in/out.

Sharding (8 cores, tensor-parallel over heads):
  core c: Q heads 4c..4c+3 (Wq cols c*512:+512), KV head c (Wk/Wv cols
  c*128:+128), Wo rows c*512:+512.  Each core computes a partial
  y^T [4096, 2048] in fp16; host sums the 8 partials and transposes.

Design (projection/out-proj matmuls bf16, attention probabilities fp16,
PSUM fp32 accumulate; measured ~392us vs 681us fp32r baseline):
  - Host pre-transposes hidden_states -> hst [4096, 2048] bf16, so H^T
    tiles stream straight from DRAM (no PE transposes, no ACT copies),
    and precomputes the RoPE cos / +-sin tables [128, 2048] fp32.
  - Wq/Wk/Wv/Wo all SBUF-resident in bf16, loaded once in 0.5MB batches.
  - PE warmup transposes at t=0 (gated only on an ACT memzero) lift the
    HAM clock gate to 2.4 GHz while the first DMAs are in flight.
  - Phase P: Q^T/K^T/V^T = W^T @ H^T accumulated over 32 hid k-tiles
    (H^T streamed in 4-k-tile chunks); RoPE on the PSUM->SBUF drain.
    The q0 accumulator is double-buffered across s-groups and the six
    PSUM drains are ordered/spread over DVE/ACT so the next s-group's
    first matmuls get their banks back just in time.  The last s-group's
    RoPE math is deferred into the attention section so the P-pool close
    barrier doesn't serialize on it.
  - Phase A per (head, q-group of 512): S^T blocks [k,q] = K^T.T @ Q^T
    pipelined 3 deep, exp on ACT (scale fused, fp16 out), causal
    diagonal blocks trimmed to the live column range in the S matmul,
    exp, gpsimd affine_select mask, fp16 DVE row-sum accumulation and
    the PV matmul alike; denominators via one ones^T matmul -> [1,512],
    reciprocal_approx_fast, gpsimd partition_broadcast.  Group 0 (no
    out-proj filler available) interleaves head pairs at j-block
    granularity to hide the exp chain latency.
  - Phase O: y^T = Wo^T @ O^T accumulated over the 4 head tiles, fp16
    out; group g's out-proj matmuls are interleaved into group g+1's
    attention ahead of each j-block so the in-order PE queue never
    starves on the exp chain.
"""
import math
import os

import numpy as np

import concourse.bass as bass
import concourse.mybir as mybir
import concourse.tile as tile
from concourse import bacc
from concourse.bass_utils import run_bass_kernel_spmd

F32 = mybir.dt.float32
F32R = mybir.dt.float32r
BF16 = mybir.dt.bfloat16
F16 = mybir.dt.float16
I32 = mybir.dt.int32
AF = mybir.ActivationFunctionType
ALU = mybir.AluOpType

T = 2048
HID = 4096
NH = 4            # q heads per core
D = 128           # head dim
DQ = NH * D       # 512
G = 512           # seq group size
NG = T // G       # 4
KT = HID // 128   # 32 hidden k-tiles
CH = 4            # k-tiles per H^T stream chunk
NCORES = 8
PD = 3            # attention S-block software pipeline depth

SCALE = 1.0 / math.sqrt(D)

LAST_EXEC_NS = None


def _emit(nc):
    hst = nc.dram_tensor("hst", [HID, T], BF16, kind="ExternalInput").ap()
    wq = nc.dram_tensor("wq", [HID, DQ], BF16, kind="ExternalInput").ap()
    wk = nc.dram_tensor("wk", [HID, D], BF16, kind="ExternalInput").ap()
    wv = nc.dram_tensor("wv", [HID, D], BF16, kind="ExternalInput").ap()
    wo = nc.dram_tensor("wo", [DQ, HID], BF16, kind="ExternalInput").ap()
    cosd = nc.dram_tensor("cosd", [128, T], F32, kind="ExternalInput").ap()
    sind = nc.dram_tensor("sind", [128, T], F32, kind="ExternalInput").ap()
    idhd = nc.dram_tensor("idhd", [128, 128], F16, kind="ExternalInput").ap()
    mskd = nc.dram_tensor("mskd", [128, 128], F16, kind="ExternalInput").ap()
    onesd = nc.dram_tensor("onesd", [128, 1], F16, kind="ExternalInput").ap()
    yt = nc.dram_tensor("yt", [HID, T], F16, kind="ExternalOutput").ap()

    hst_k = hst.rearrange("(a p) t -> p a t", p=128)   # [128, KT, T]
    wq_k = wq.rearrange("(a p) m -> p a m", p=128)     # [128, KT, DQ]
    wk_k = wk.rearrange("(a p) m -> p a m", p=128)     # [128, KT, D]
    wv_k = wv.rearrange("(a p) m -> p a m", p=128)

    with tile.TileContext(nc) as tc:
        with (
            tc.tile_pool(name="const", bufs=1) as const,
            tc.tile_pool(name="res", bufs=1) as res,
            tc.tile_pool(name="hp", bufs=6) as hp,
            tc.tile_pool(name="ro", bufs=2) as ro,
            tc.tile_pool(name="ex", bufs=8) as ex,
            tc.tile_pool(name="sc", bufs=2) as sc,
            tc.tile_pool(name="yo", bufs=4) as yo,
        ):
            # ------------- constants -------------
            # idf only feeds the PE warmup spin and the dummy broadcast, so
            # its values are don't-care: a DVE memset (ready ~5us, earlier
            # than the ~6-8.5us gpsimd boot) gates the warmup.  The real
            # identity / causal mask / ones constants ride input DMAs
            # (data path wakes ~9us, well before their first use).
            idf = const.tile([128, 128], F32, name="idf", tag="idf")
            nc.vector.memset(idf[:], 1.0)
            # tiles only; their DMAs are emitted mid-stream in phase P so
            # the head of the (FIFO) HWDGE rings is all first-matmul data
            identh = const.tile([128, 128], F16, name="identh", tag="identh")
            maskt = const.tile([128, 128], F16, name="maskt", tag="maskt")
            ones = const.tile([128, 1], F16, name="ones", tag="ones")

            # dummy partition_broadcast: forces the gpsimd "attn" library
            # load (~7us, observed as LIBRARY_RELOAD) to happen here, under
            # phase P, instead of stalling the first softmax normalize
            pbw = const.tile([128, 8], F32, name="pbw", tag="pbw")
            nc.gpsimd.partition_broadcast(pbw[:], idf[0:1, 0:8])

            # resident weights and rope tables
            wq_sb = res.tile([128, KT, DQ], BF16, name="wq_sb", tag="wq_sb")
            wk_sb = res.tile([128, KT, D], BF16, name="wk_sb", tag="wk_sb")
            wv_sb = res.tile([128, KT, D], BF16, name="wv_sb", tag="wv_sb")
            wo_sb = res.tile([128, NH, HID], BF16, name="wo_sb", tag="wo_sb")
            cosf = res.tile([128, T], F32, name="cosf", tag="cosf")
            sinpm = res.tile([128, T], F32, name="sinpm", tag="sinpm")

            # resident activations (qt also doubles as O^T after attention)
            qt = [res.tile([128, T], BF16, name=f"qt{h}", tag=f"qt{h}")
                  for h in range(NH)]
            kt = res.tile([128, T], BF16, name="kt", tag="kt")
            vnat = res.tile([128, T // 128, D], F16, name="vnat", tag="vnat")

            # ---------------- phase P: projections ----------------
            with (
                tc.tile_pool(name="accp", bufs=1, space="PSUM") as accp,
                tc.tile_pool(name="tpp", bufs=1, space="PSUM") as tpp,
            ):
                # spin the PE while the first DMAs land so HAM unthrottles;
                # source is idf (gpsimd boots faster than ACT), target is one
                # buffer of the double-buffered qps0 ring
                warm_tp = accp.tile([128, G], F32, name="qps0", tag="qps0",
                                    bufs=2)
                for _ in range(24):
                    nc.tensor.transpose(warm_tp[:, 0:128], idf[:], idf[:])

                for s in range(NG):
                    ssl = bass.ts(s, G)
                    q_ps = [accp.tile([128, G], F32, name=f"qps{f}",
                                      tag=f"qps{f}",
                                      bufs=2 if f == 0 else 1)
                            for f in range(NH)]
                    k_ps = accp.tile([128, G], F32, name="kps", tag="kps")
                    v_ps = accp.tile([128, G], F32, name="vps", tag="vps")

                    for c in range(KT // CH):
                        csl2 = bass.ds(c * CH, CH)
                        first = (s == 0 and c == 0)
                        htc = hp.tile([128, CH, G], BF16, name="htc",
                                      tag="htc")
                        if first:
                            # k=0..1 ride the (otherwise idle) scalar HWDGE
                            # ring while k=2..3 and the bulk stream use the
                            # sync ring: both rings' FIFO heads carry
                            # first-needed data in parallel, so the first
                            # matmul gates on ~350KB instead of ~1MB
                            nc.scalar.dma_start(htc[:, 0:1, :],
                                                hst_k[:, 0:1, ssl])
                            nc.scalar.dma_start(wq_sb[:, 0:1, :],
                                                wq_k[:, 0:1, :])
                            nc.scalar.dma_start(wk_sb[:, 0:2, :],
                                                wk_k[:, 0:2, :])
                            nc.scalar.dma_start(wv_sb[:, 0:2, :],
                                                wv_k[:, 0:2, :])
                            nc.scalar.dma_start(htc[:, 1:2, :],
                                                hst_k[:, 1:2, ssl])
                            nc.scalar.dma_start(wq_sb[:, 1:2, :],
                                                wq_k[:, 1:2, :])
                            nc.sync.dma_start(wq_sb[:, 2:4, :],
                                              wq_k[:, 2:4, :])
                            nc.sync.dma_start(htc[:, 2:4, :],
                                              hst_k[:, 2:4, ssl])
                            nc.sync.dma_start(wk_sb[:, 2:4, :],
                                              wk_k[:, 2:4, :])
                            nc.sync.dma_start(wv_sb[:, 2:4, :],
                                              wv_k[:, 2:4, :])
                        elif s == 0:
                            # alternate chunks across the two HWDGE rings:
                            # each ring then streams ~2.6MB of s0 data at
                            # ~2.7us/chunk against a 5.1us/chunk burn rate
                            eng = nc.scalar if c % 2 == 1 else nc.sync
                            eng.dma_start(wq_sb[:, csl2, :],
                                          wq_k[:, csl2, :])
                            eng.dma_start(htc[:], hst_k[:, csl2, ssl])
                            eng.dma_start(wk_sb[:, csl2, :],
                                          wk_k[:, csl2, :])
                            eng.dma_start(wv_sb[:, csl2, :],
                                          wv_k[:, csl2, :])
                            if c == 6:
                                # small consts (land mid-s0, used ~45us)
                                nc.sync.dma_start(identh[:], idhd[:, :])
                                nc.sync.dma_start(maskt[:], mskd[:, :])
                                nc.sync.dma_start(ones[:], onesd[:, :])
                            if c == 7:
                                nc.sync.dma_start(cosf[:], cosd[:, :])
                                nc.scalar.dma_start(sinpm[:], sind[:, :])
                        else:
                            nc.scalar.dma_start(htc[:],
                                                hst_k[:, csl2, ssl])
                        for kk in range(CH):
                            k = c * CH + kk
                            ht = htc[:, kk, :]
                            st = (k == 0)
                            sp = (k == KT - 1)
                            # bank order must match the drain schedule below
                            nc.tensor.matmul(q_ps[0][:], wq_sb[:, k, 0:128],
                                             ht, start=st, stop=sp)
                            nc.tensor.matmul(k_ps[:], wk_sb[:, k, :], ht,
                                             start=st, stop=sp)
                            nc.tensor.matmul(v_ps[:], wv_sb[:, k, :], ht,
                                             start=st, stop=sp)
                            nc.tensor.matmul(q_ps[2][:],
                                             wq_sb[:, k, 256:384],
                                             ht, start=st, stop=sp)
                            nc.tensor.matmul(q_ps[3][:],
                                             wq_sb[:, k, 384:512],
                                             ht, start=st, stop=sp)
                            nc.tensor.matmul(q_ps[1][:],
                                             wq_sb[:, k, 128:256],
                                             ht, start=st, stop=sp)

                    # drain all six accumulators first, on the engines and
                    # in the order the next s-group's matmuls reuse banks:
                    # q0 is double-buffered (no rush), then k,v,q2,q3 on
                    # DVE (267ns each), q1 then q0 on ACT.
                    raws = {}
                    if s == NG - 1:
                        # no next s-group: the V transposes (and the whole
                        # in-order PE queue into attention behind them) gate
                        # on vraw — drain it first.  Then drain in the order
                        # the A-phase PSUM pools reuse the banks (pss gets
                        # the q0/q1 banks for the first S tiles, pso gets
                        # q3/k for the first PV accumulators).
                        drain_order = ((NH + 1, 'v'), (0, 's'), (1, 'v'),
                                       (3, 's'), (NH, 'v'), (2, 's'))
                    else:
                        drain_order = ((NH, 'v'), (1, 'v'), (2, 'v'),
                                       (3, 'v'), (NH + 1, 'v'), (0, 's'))
                    for x, eng in drain_order:
                        src = (q_ps[x] if x < NH else
                               (k_ps if x == NH else v_ps))
                        if x == NH + 1:
                            vraw = ro.tile([128, G], F16, name="vraw",
                                           tag="vraw", bufs=1)
                            nc.vector.tensor_copy(vraw[:], src[:])
                            continue
                        raw = ro.tile([128, G], F32, name="raw", tag="raw",
                                      bufs=6)
                        if eng == 'v':
                            nc.vector.tensor_copy(raw[:], src[:])
                        else:
                            nc.scalar.copy(raw[:], src[:])
                        raws[x] = raw

                    # v: PE-transpose to natural layout
                    tpv = tpp.tile([128, G], F16, name="tph", tag="tph")
                    for sub in range(4):
                        nc.tensor.transpose(
                            tpv[:, sub * 128:(sub + 1) * 128],
                            vraw[:, sub * 128:(sub + 1) * 128], identh[:])
                    nc.scalar.copy(vnat[:, 4 * s:4 * s + 4, :], tpv[:])

                    # RoPE for q heads + k.  The last s-group's rope output
                    # is only read by attention group 3, ~150us later — defer
                    # its emission into the attention section so the P-pool
                    # close barrier (and attention group 0 behind it) doesn't
                    # wait out the 15-op DVE chain.
                    def emit_rope(x, raws=raws, ssl=ssl):
                        raw = raws[x]
                        dst = qt[x][:, ssl] if x < NH else kt[:, ssl]
                        rot = ro.tile([128, G], F32, name="rot", tag="rot")
                        nc.sync.dma_start(rot[0:64, :], raw[64:128, :])
                        nc.sync.dma_start(rot[64:128, :], raw[0:64, :])
                        tmp = ro.tile([128, G], F32, name="tmp", tag="tmp",
                                      bufs=1)
                        nc.vector.tensor_mul(tmp[:], rot[:], sinpm[:, ssl])
                        nc.vector.tensor_mul(dst, raw[:], cosf[:, ssl])
                        nc.vector.tensor_add(dst, dst, tmp[:])

                    if s == NG - 1:
                        rope_s3 = [lambda x=x: emit_rope(x)
                                   for x in range(NH + 1)]
                    else:
                        for x in range(NH + 1):
                            emit_rope(x)

            # out-proj weights: after P the scalar ring is idle (htc stream
            # done) and the sync ring still carries rope + y-out DMAs, so
            # the 4MB wo load rides scalar; needed only at first Y phase.
            nc.scalar.dma_start(wo_sb[:],
                                wo.rearrange("(f p) j -> p f j", p=128))

            # ---------------- phase A + O interleaved ----------------
            with (
                tc.tile_pool(name="pss", bufs=3, space="PSUM") as pss,
                tc.tile_pool(name="pssum", bufs=1, space="PSUM") as pssum,
                tc.tile_pool(name="pso", bufs=2, space="PSUM") as pso,
                tc.tile_pool(name="psy", bufs=2, space="PSUM") as psy,
            ):
                def emit_y(gy, m, pool=None, tagn="yps"):
                    gsl = bass.ts(gy, G)
                    y_ps = (pool or psy).tile([128, G], F32, name=tagn,
                                              tag=tagn)
                    for f in range(NH):
                        nc.tensor.matmul(
                            y_ps[:], wo_sb[:, f, m * 128:(m + 1) * 128],
                            qt[f][:, gsl],
                            start=(f == 0), stop=(f == NH - 1))
                    y_sb = yo.tile([128, G], F16, name="ysb", tag="ysb")
                    # split drain: halves on both engines in parallel frees
                    # the PSUM bank ~350ns sooner than one 690ns copy
                    nc.scalar.copy(y_sb[:, 0:256], y_ps[:, 0:256])
                    nc.vector.tensor_copy(y_sb[:, 256:512], y_ps[:, 256:512])
                    nc.sync.dma_start(yt[m * 128:(m + 1) * 128, gsl], y_sb[:])

                # Deferred softmax finalization: the denominator matmul for
                # head h rides the in-order PE queue, so emitting it right
                # after h's last PV matmul stalls the PE on the DVE sum
                # chain.  Instead each head's finalize (den MM, reciprocal,
                # broadcast, normalize multiply) is queued and flushed after
                # the NEXT head's S-matmul prologue, giving the PE
                # dependency-free runway while the chains complete.
                pending = []

                def flush_pending(n=None):
                    k = len(pending) if n is None else min(n, len(pending))
                    for _ in range(k):
                        pending.pop(0)()

                def make_fin(h, gsl, o_ps, sumacc, pop_rope):
                    def fin():
                        s_sum = pssum.tile([1, G], F32, name="ssum",
                                           tag="ssum")
                        nc.tensor.matmul(s_sum[:], ones[:], sumacc[:],
                                         start=True, stop=True)
                        s_rec = sc.tile([1, G], F32, name="srec", tag="srec")
                        nc.vector.reciprocal_approx_fast(s_rec[:], s_sum[:])
                        recb = sc.tile([128, G], F32, name="recb", tag="recb")
                        nc.gpsimd.partition_broadcast(recb[:], s_rec[:])
                        nc.vector.tensor_mul(qt[h][:, gsl], o_ps[:], recb[:])
                        if pop_rope and rope_s3:
                            rope_s3.pop(0)()
                    return fin

                # ---- group 0: no out-proj filler exists yet, so hide the
                # exp->mask chain latency by interleaving ALL FOUR heads at
                # j-block granularity (four independent chains in flight;
                # the idle out-proj banks hold two of the four O accums).
                gsl0 = bass.ts(0, G)
                order = [(h, j) for j in range(4) for h in range(NH)]
                o_ps0 = {}
                sum0 = {}
                for h in range(NH):
                    if h < 2:
                        o_ps0[h] = pso.tile([128, G], F32, name="ops",
                                            tag="ops")
                    else:
                        o_ps0[h] = psy.tile([128, G], F32, name="yps",
                                            tag="yps")
                    sum0[h] = sc.tile([128, G], F16, name="sumacc",
                                      tag="sumacc", bufs=6)
                st0 = {}

                def emit_s0(h, j):
                    s_ps = pss.tile([128, G], F32, name="sps", tag="sps")
                    qs = j * 128
                    nc.tensor.matmul(
                        s_ps[:, qs:], kt[:, j * 128:(j + 1) * 128],
                        qt[h][:, qs:G], start=True, stop=True)
                    st0[(h, j)] = s_ps

                for i in range(PD):
                    emit_s0(*order[i])
                for i, (h, j) in enumerate(order):
                    s_ps = st0.pop((h, j))
                    qs = j * 128
                    e_sb = ex.tile([128, G], F16, name="esb", tag="esb")
                    nc.scalar.activation(e_sb[:, qs:], s_ps[:, qs:],
                                         AF.Exp, scale=SCALE)
                    # masks always on DVE (134ns f16 multiply): keeps the
                    # strict-FIFO gpsimd queue free for the fin broadcasts
                    nc.vector.tensor_mul(e_sb[:, qs:qs + 128],
                                         e_sb[:, qs:qs + 128],
                                         maskt[:])
                    if i + PD < len(order):
                        emit_s0(*order[i + PD])
                    if j == 0:
                        nc.vector.tensor_copy(sum0[h][:], e_sb[:])
                    else:
                        # masked columns [:qs] are exact zeros — skip them
                        nc.vector.tensor_add(sum0[h][:, qs:],
                                             sum0[h][:, qs:],
                                             e_sb[:, qs:])
                    nc.tensor.matmul(o_ps0[h][:, qs:], vnat[:, j, :],
                                     e_sb[:, qs:],
                                     start=(j == 0), stop=(j == 3))
                for h in range(NH):
                    pending.append(make_fin(h, gsl0, o_ps0[h], sum0[h],
                                            False))

                for g in range(1, NG):
                    jn = 4 * g + 4
                    tj = NH * jn       # attention j-blocks in this group
                    jdone = 0
                    ym = 0             # Y_{g-1} m-tiles emitted so far
                    for h in range(NH):
                        gsl = bass.ts(g, G)
                        o_ps = pso.tile([128, G], F32, name="ops", tag="ops")
                        # softmax denominators accumulate in fp16 on the
                        # DVE (2x 16-bit rate, 10 mantissa bits keeps the
                        # accumulation error ~0.1%); one final ones-matmul
                        # reduces over partitions.
                        sumacc = sc.tile([128, G], F16, name="sumacc",
                                         tag="sumacc", bufs=6)

                        s_tiles = {}

                        def emit_s(j, h=h, g=g):
                            s_ps = pss.tile([128, G], F32, name="sps",
                                            tag="sps")
                            qs = max(0, (j - 4 * g) * 128)
                            nc.tensor.matmul(
                                s_ps[:, qs:], kt[:, j * 128:(j + 1) * 128],
                                qt[h][:, g * G + qs:(g + 1) * G],
                                start=True, stop=True)
                            s_tiles[j] = s_ps

                        for j in range(min(PD, jn)):
                            emit_s(j)
                        # one fin per flush point: consecutive den matmuls
                        # would serialize the in-order PE queue on the
                        # single pssum bank (den(h+1) waits recip(h) read);
                        # group 0's four-fin backlog drains one per j-block
                        flush_pending(1)
                        # group 1 starts its Y interleave only after the
                        # group-0 fin backlog has fully drained
                        dly = 4 if g == 1 else 0
                        for j in range(jn):
                            if j >= 1:
                                flush_pending(1)
                            # out-proj of the previous group rides ahead of
                            # this j-block to keep the PE fed while ACT works
                            if g >= 1:
                                while ym < 32 and ym * tj < 32 * (jdone - dly):
                                    emit_y(g - 1, ym)
                                    ym += 1
                            s_ps = s_tiles.pop(j)
                            o = j - 4 * g
                            qs = max(0, o * 128)
                            e_sb = ex.tile([128, G], F16, name="esb",
                                           tag="esb")
                            nc.scalar.activation(e_sb[:, qs:], s_ps[:, qs:],
                                                 AF.Exp, scale=SCALE)
                            if o >= 0:
                                # causal mask: only the 128-wide diagonal
                                # triangle can fail col >= p + qs; DVE
                                # multiply keeps gpsimd free for broadcasts
                                nc.vector.tensor_mul(e_sb[:, qs:qs + 128],
                                                     e_sb[:, qs:qs + 128],
                                                     maskt[:])
                            if j + PD < jn:
                                emit_s(j + PD)
                            if j == 0:
                                nc.vector.tensor_copy(sumacc[:], e_sb[:])
                            else:
                                # masked cols [:qs] are exact zeros — skip
                                nc.vector.tensor_add(sumacc[:, qs:],
                                                     sumacc[:, qs:],
                                                     e_sb[:, qs:])
                            nc.tensor.matmul(o_ps[:, qs:], vnat[:, j, :],
                                             e_sb[:, qs:],
                                             start=(j == 0), stop=(j == jn - 1))
                            jdone += 1
                        # ones^T @ sumacc -> [1, G] denominators on the PE,
                        # then fast-approx reciprocal (~18 bits, plenty);
                        # deferred past the next head's S prologue.
                        pending.append(make_fin(h, gsl, o_ps, sumacc, True))
                    # flush the last head's finalize before the trailing
                    # out-proj tiles: the trailing Y matmuls (independent of
                    # this group's output) give the den->recip->broadcast->
                    # mul chain PE-free runway
                    flush_pending()
                    while ym < 32:
                        emit_y(g - 1, ym)
                        ym += 1
                # final group's out-projection; rotate accumulators over all
                # three now-idle PSUM pools (psy/pss/pso) so the drain of
                # tile m never gates the matmuls of tile m+2
                for m in range(KT):
                    pool, tagn = ((psy, "yps"), (pss, "sps"),
                                  (pso, "ops"))[m % 3]
                    emit_y(NG - 1, m, pool, tagn)
    return nc


_NC_CACHE = None


def _get_nc():
    global _NC_CACHE
    if _NC_CACHE is None:
        nc = bacc.Bacc("TRN2", target_bir_lowering=False, debug=False,
                       num_devices=NCORES)
        _emit(nc)
        nc.compile()
        _NC_CACHE = nc
    return _NC_CACHE


def _install_ntff_hook():
    import sys
    import types
    try:
        import trn_agent_boot.trn_boot as tb
        hook = tb._ntff_profile_via_ctypes('/opt/axon/libaxon_pjrt.so')
        if hook is None:
            return
        mod = types.ModuleType('antenv.axon_hooks')
        mod.get_axon_ntff_profile_hook = lambda: hook
        sys.modules['antenv.axon_hooks'] = mod
    except Exception:
        pass


def kernel(**inputs):
    global LAST_EXEC_NS
    import ml_dtypes
    BF = ml_dtypes.bfloat16

    positions = np.asarray(inputs["positions"]).astype(np.float64)
    hidden = np.asarray(inputs["hidden_states"], dtype=np.float32)
    Wq = np.asarray(inputs["Wq"], dtype=np.float32)
    Wk = np.asarray(inputs["Wk"], dtype=np.float32)
    Wv = np.asarray(inputs["Wv"], dtype=np.float32)
    Wo = np.asarray(inputs["Wo"], dtype=np.float32)

    hst = np.ascontiguousarray(hidden.T).astype(BF)

    # neox rotate-half RoPE tables, partition p carries frequency p & 63;
    # top half gets -sin so that raw*cos + rot*sinpm == rotate_half rope.
    p = np.arange(128)
    invf = (1e6) ** (-(p & 63) / 64.0)
    ang = invf[:, None] * positions[None, :]
    cosd = np.cos(ang).astype(np.float32)
    sin = np.sin(ang)
    sind = np.concatenate([-sin[:64], sin[64:]], axis=0).astype(np.float32)

    trace = os.environ.get("KERNEL_TRACE", "0") == "1"
    if trace:
        _install_ntff_hook()

    idhd = np.eye(128, dtype=np.float16)
    mskd = np.triu(np.ones((128, 128), dtype=np.float16))
    onesd = np.ones((128, 1), dtype=np.float16)

    nc = _get_nc()
    in_maps = []
    for c in range(NCORES):
        in_maps.append({
            "hst": hst,
            "wq": np.ascontiguousarray(Wq[:, c * DQ:(c + 1) * DQ]).astype(BF),
            "wk": np.ascontiguousarray(Wk[:, c * D:(c + 1) * D]).astype(BF),
            "wv": np.ascontiguousarray(Wv[:, c * D:(c + 1) * D]).astype(BF),
            "wo": np.ascontiguousarray(Wo[c * DQ:(c + 1) * DQ, :]).astype(BF),
            "cosd": cosd,
            "sind": sind,
            "idhd": idhd,
            "mskd": mskd,
            "onesd": onesd,
        })
    res = run_bass_kernel_spmd(nc, in_maps, core_ids=list(range(NCORES)),
                               trace=trace)
    LAST_EXEC_NS = res.exec_time_ns
    acc = np.zeros((HID, T), dtype=np.float32)
    for c in range(NCORES):
        acc += res.results[c]["yt"].astype(np.float32)
    return np.ascontiguousarray(acc.T)

